# revision 6
# baseline (speedup 1.0000x reference)
"""Trainium2 Bass kernel for nn_CrossAttnBlock (sparse_attention, memory-bound).

Math note: in the reference, the attention logits are broadcast along the
*key* axis before the softmax, so the softmax runs over a constant vector
and is exactly uniform (1/(H*W)).  The attention output therefore collapses
to v broadcast over space, and the whole block reduces to

    out[b,c,h,w] = x[b,c,h,w] + (w3 @ (w2 @ context[b] + b2) + b3)[c]

GroupNorm / q / k are dead code.  Folding the weights host-side
(Wf = w3 @ w2, bf = w3 @ b2 + b3 -- input-independent constant folding)
reduces the device work to

    proj[b] = Wf @ context[b] + bf          (tiny matvec, tensor engine)
    out     = x + proj[b][c]                (memory-bound stream)

Sharding: pure data parallel over batch (B=8 -> 1 batch element per core);
folded params replicated on every core.

Performance notes (from the baseline trace):
  * Each SWDGE dma_start costs ~650ns of *serial* gpsimd descriptor
    generation, so the kernel uses only 5 triggers: pack, x lo/hi,
    out lo/hi.  Emission order on the single SWDGE ring IS the transfer
    schedule; with this order the ring never idles.
  * The x stream runs in bf16 (in and out), halving the dominant HBM
    traffic.  absmax error ~1e-2 * max|x| * 2^-9 ~ 0.02, far inside the
    2e-2 relative-error gate.
  * All matvec constants AND the per-core context ride in ONE per-core
    DRAM tensor: walrus allows only one sync-wait on a Matmult (it rides
    the LoadWeights slot), so the first matmul may depend on at most one
    DMA queue.
"""

import numpy as np
import ml_dtypes

import concourse.bass as bass
import concourse.bacc as bacc
import concourse.tile as tile
from concourse import mybir
from concourse.bass_utils import run_bass_kernel_spmd

N_CORES = 8
B, C, H, W, CC = 8, 256, 48, 48, 512
S = H * W              # 2304 spatial positions
P = 128                # SBUF partitions
CI = C // P            # 2 channel chunks (channel = ci*128 + p)
KJ = CC // P           # 4 contraction chunks (k = 4*p + j)

# pack layout, bf16 [P, PACK_COLS]:
#   cols [ (j*CI+oi)*P : +P ] : WfT block  (p, m) = Wf[oi*P+m, KJ*p+j]
#   OFF_CTX + j              : ctx        (p)    = context[KJ*p+j]
#   OFF_BIAS + oi            : bias       (p)    = bf[oi*P+p]   (bf16)
OFF_CTX = KJ * CI * P          # 1024
OFF_BIAS = OFF_CTX + KJ        # 1028
PACK_COLS = OFF_BIAS + CI      # 1030

_F32 = mybir.dt.float32
_BF16 = mybir.dt.bfloat16
BF = ml_dtypes.bfloat16


def build_nc(loop_r: int = 1, splits: int = 2) -> bass.Bass:
    # Bacc (not raw Bass): its finalize pipeline runs generate_event_semaphores,
    # which splits multi-waits -- TRN2 allows at most 1 sync wait per instruction.
    nc = bacc.Bacc()

    x_d = [nc.dram_tensor(f"x{ci}", [P, S], _BF16, kind="ExternalInput")
           for ci in range(CI)]
    pk_d = nc.dram_tensor("pack", [P, PACK_COLS], _BF16, kind="ExternalInput")
    out_d = [nc.dram_tensor(f"out{ci}", [P, S], _BF16, kind="ExternalOutput")
             for ci in range(CI)]

    with tile.TileContext(nc) as tc:
        with (
            tc.tile_pool(name="consts", bufs=1) as consts,
            tc.tile_pool(name="small", bufs=1) as small,
            tc.tile_pool(name="psum", bufs=1, space="PSUM") as psum,
            tc.tile_pool(name="stream", bufs=1) as stream,
        ):
            for _ in range(loop_r):
                # One SWDGE trigger for every constant + ctx (single DMA
                # queue -> the first matmul carries a single sync wait).
                pk = consts.tile([P, PACK_COLS], _BF16, tag="pk")
                nc.gpsimd.dma_start(out=pk, in_=pk_d[:])

                # x stream enters the ring right behind the pack, split so
                # each chunk's add (and then its out trigger) fires as soon
                # as that chunk lands: the SWDGE ring never idles between
                # the in-phase and the out-phase.  gpsimd descriptor-gen
                # (~0.65us per trigger) overlaps transfers; with 1+2*splits
                # triggers it stays ahead of the ring.
                fc = S // splits
                xt = []
                for ci in range(CI):
                    t = stream.tile([P, S], _BF16, tag=f"x{ci}")
                    xt.append(t)
                    for q in range(splits):
                        sl = bass.ts(q, fc)
                        nc.gpsimd.dma_start(out=t[:, sl], in_=x_d[ci][:, sl])

                # proj[oi*P+m] = sum_k Wf[oi*P+m, k] * ctx[k], k = 4p+j.
                # 8 tiny bf16 matmuls straight off the pack DMA.
                pp = psum.tile([P, CI], _F32, tag="pp")
                for oi in range(CI):
                    for j in range(KJ):
                        blk = (j * CI + oi) * P
                        nc.tensor.matmul(
                            pp[:, oi : oi + 1],
                            lhsT=pk[:, blk : blk + P],
                            rhs=pk[:, OFF_CTX + j : OFF_CTX + j + 1],
                            start=(j == 0),
                            stop=(j == KJ - 1),
                        )
                proj = small.tile([P, CI], _F32, tag="proj")
                nc.vector.tensor_add(proj, pp, pk[:, OFF_BIAS : OFF_BIAS + CI])

                # out = x + proj, per chunk: add as soon as the chunk's DMA
                # lands, then its out trigger rides the ring right behind
                # the remaining in-chunks.
                for ci in range(CI):
                    for q in range(splits):
                        sl = bass.ts(q, fc)
                        nc.vector.tensor_scalar_add(
                            xt[ci][:, sl], xt[ci][:, sl], proj[:, ci : ci + 1]
                        )
                        nc.gpsimd.dma_start(out=out_d[ci][:, sl], in_=xt[ci][:, sl])

    nc.finalize()
    return nc


def _prep_in_maps(inputs: dict) -> list[dict]:
    f32 = lambda a: np.ascontiguousarray(np.asarray(a), dtype=np.float32)
    x = f32(inputs["x"])                    # [B, C, H, W]
    context = f32(inputs["context"])        # [B, CC]
    w2 = f32(inputs["w2"])                  # [C, CC]
    b2 = f32(inputs["b2"])                  # [C]
    w3 = f32(inputs["w3"])                  # [C, C]
    b3 = f32(inputs["b3"])                  # [C]

    wf = w3 @ w2                            # [C, CC] folded weight
    bf = w3 @ b2 + b3                       # [C]     folded bias

    # WfT blocks: pack[p, (j*CI+oi)*P + m] = Wf[oi*P+m, KJ*p+j]
    wft = wf.T.reshape(P, KJ, CI, P).transpose(0, 1, 2, 3)  # [p, j, oi, m]
    pack = np.zeros((P, PACK_COLS), dtype=BF)
    pack[:, : KJ * CI * P] = wft.reshape(P, KJ * CI * P).astype(BF)
    pack[:, OFF_BIAS : OFF_BIAS + CI] = bf.reshape(CI, P).T.astype(BF)

    xb = x.reshape(B, CI, P, S).astype(BF)  # channel = ci*128 + p

    in_maps = []
    for b in range(N_CORES):
        m = {f"x{ci}": xb[b, ci] for ci in range(CI)}
        pkb = pack.copy()
        pkb[:, OFF_CTX : OFF_CTX + KJ] = context[b].reshape(P, KJ).astype(BF)
        m["pack"] = pkb
        in_maps.append(m)
    return in_maps


def run(inputs: dict, trace: bool = False, tmpdir: str | None = None, **build_kw):
    """Build+run on 8 cores; returns (full_output, BassKernelResults)."""
    nc = build_nc(**build_kw)
    in_maps = _prep_in_maps(inputs)
    res = run_bass_kernel_spmd(
        nc, in_maps, list(range(N_CORES)), trace=trace, tmpdir=tmpdir
    )
    out = np.stack(
        [
            np.concatenate(
                [res.results[b][f"out{ci}"] for ci in range(CI)], axis=0
            ).astype(np.float32)
            for b in range(N_CORES)
        ],
        axis=0,
    ).reshape(B, C, H, W)
    return out, res


def kernel(**inputs: np.ndarray) -> np.ndarray:
    out, _ = run(inputs, trace=False)
    return out


# revision 9
# speedup vs baseline: 1.1147x; 1.1147x over previous
"""Trainium2 Bass kernel for nn_CrossAttnBlock (sparse_attention, memory-bound).

Math note: in the reference, the attention logits are broadcast along the
*key* axis before the softmax, so the softmax runs over a constant vector
and is exactly uniform (1/(H*W)).  The attention output therefore collapses
to v broadcast over space, and the whole block reduces to

    out[b,c,h,w] = x[b,c,h,w] + (w3 @ (w2 @ context[b] + b2) + b3)[c]

GroupNorm / q / k are dead code.  Folding the weights host-side
(Wf = w3 @ w2, bf = w3 @ b2 + b3 -- input-independent constant folding)
reduces the device work to

    proj[b] = Wf @ context[b] + bf          (tiny matvec, tensor engine)
    out     = x + proj[b][c]                (memory-bound stream)

Sharding: pure data parallel over batch (B=8 -> 1 batch element per core);
folded params replicated on every core.

Performance notes (from the baseline trace):
  * Each SWDGE dma_start costs ~650ns of *serial* gpsimd descriptor
    generation, so the kernel uses only 5 triggers: pack, x lo/hi,
    out lo/hi.  Emission order on the single SWDGE ring IS the transfer
    schedule; with this order the ring never idles.
  * The x stream runs in bf16 (in and out), halving the dominant HBM
    traffic.  absmax error ~1e-2 * max|x| * 2^-9 ~ 0.02, far inside the
    2e-2 relative-error gate.
  * All matvec constants AND the per-core context ride in ONE per-core
    DRAM tensor: walrus allows only one sync-wait on a Matmult (it rides
    the LoadWeights slot), so the first matmul may depend on at most one
    DMA queue.
"""

import numpy as np
import ml_dtypes

import concourse.bass as bass
import concourse.bacc as bacc
import concourse.tile as tile
from concourse import mybir
from concourse.bass_utils import run_bass_kernel_spmd

N_CORES = 8
B, C, H, W, CC = 8, 256, 48, 48, 512
S = H * W              # 2304 spatial positions
P = 128                # SBUF partitions
CI = C // P            # 2 channel chunks (channel = ci*128 + p)
KJ = CC // P           # 4 contraction chunks (k = 4*p + j)

# pack layout, bf16 [P, PACK_COLS]:
#   cols [ (j*CI+oi)*P : +P ] : WfT block  (p, m) = Wf[oi*P+m, KJ*p+j]
#   OFF_CTX + j              : ctx        (p)    = context[KJ*p+j]
#   OFF_BIAS + oi            : bias       (p)    = bf[oi*P+p]   (bf16)
OFF_CTX = KJ * CI * P          # 1024
OFF_BIAS = OFF_CTX + KJ        # 1028
PACK_COLS = OFF_BIAS + CI      # 1030

_F32 = mybir.dt.float32
_BF16 = mybir.dt.bfloat16
BF = ml_dtypes.bfloat16


def build_nc(loop_r: int = 1, splits: int = 1, hwdge_out: bool = True,
             hwdge_pack: bool = True) -> bass.Bass:
    # Bacc (not raw Bass): its finalize pipeline runs generate_event_semaphores,
    # which splits multi-waits -- TRN2 allows at most 1 sync wait per instruction.
    nc = bacc.Bacc()

    x_d = [nc.dram_tensor(f"x{ci}", [P, S], _BF16, kind="ExternalInput")
           for ci in range(CI)]
    pk_d = nc.dram_tensor("pack", [P, PACK_COLS], _BF16, kind="ExternalInput")
    out_d = [nc.dram_tensor(f"out{ci}", [P, S], _BF16, kind="ExternalOutput")
             for ci in range(CI)]

    with tile.TileContext(nc) as tc:
        with (
            tc.tile_pool(name="consts", bufs=1) as consts,
            tc.tile_pool(name="small", bufs=1) as small,
            tc.tile_pool(name="psum", bufs=1, space="PSUM") as psum,
            tc.tile_pool(name="stream", bufs=1) as stream,
        ):
            for _ in range(loop_r):
                # One trigger for every constant + ctx (single DMA queue ->
                # the first matmul carries a single sync wait).  On sync
                # HWDGE the descriptors are generated in hardware, so the
                # transfer starts ~2us before the SWDGE path would.
                pk = consts.tile([P, PACK_COLS], _BF16, tag="pk")
                if hwdge_pack:
                    nc.sync.dma_start(out=pk, in_=pk_d[:])
                else:
                    nc.gpsimd.dma_start(out=pk, in_=pk_d[:])

                # x stream enters the ring right behind the pack, split so
                # each chunk's add (and then its out trigger) fires as soon
                # as that chunk lands: the SWDGE ring never idles between
                # the in-phase and the out-phase.  gpsimd descriptor-gen
                # (~0.65us per trigger) overlaps transfers; with 1+2*splits
                # triggers it stays ahead of the ring.
                fc = S // splits
                xt = []
                for ci in range(CI):
                    t = stream.tile([P, S], _BF16, tag=f"x{ci}")
                    xt.append(t)
                    for q in range(splits):
                        sl = bass.ts(q, fc)
                        nc.gpsimd.dma_start(out=t[:, sl], in_=x_d[ci][:, sl])

                # proj[oi*P+m] = sum_k Wf[oi*P+m, k] * ctx[k], k = 4p+j.
                # 8 tiny bf16 matmuls straight off the pack DMA.
                pp = psum.tile([P, CI], _F32, tag="pp")
                for oi in range(CI):
                    for j in range(KJ):
                        blk = (j * CI + oi) * P
                        nc.tensor.matmul(
                            pp[:, oi : oi + 1],
                            lhsT=pk[:, blk : blk + P],
                            rhs=pk[:, OFF_CTX + j : OFF_CTX + j + 1],
                            start=(j == 0),
                            stop=(j == KJ - 1),
                        )
                proj = small.tile([P, CI], _F32, tag="proj")
                nc.vector.tensor_add(proj, pp, pk[:, OFF_BIAS : OFF_BIAS + CI])

                # out = x + proj, per chunk: add as soon as the chunk's DMA
                # lands, then its out DMA.  HWDGE (sync/scalar) out path:
                # no serial gpsimd descriptor-gen, and the out queues run
                # independently of the in-ring so the phases overlap.
                out_eng = [nc.sync, nc.scalar] if hwdge_out else [nc.gpsimd, nc.gpsimd]
                for ci in range(CI):
                    for q in range(splits):
                        sl = bass.ts(q, fc)
                        nc.vector.tensor_scalar_add(
                            xt[ci][:, sl], xt[ci][:, sl], proj[:, ci : ci + 1]
                        )
                        out_eng[ci].dma_start(out=out_d[ci][:, sl], in_=xt[ci][:, sl])

    nc.finalize()
    return nc


def _prep_in_maps(inputs: dict) -> list[dict]:
    f32 = lambda a: np.ascontiguousarray(np.asarray(a), dtype=np.float32)
    x = f32(inputs["x"])                    # [B, C, H, W]
    context = f32(inputs["context"])        # [B, CC]
    w2 = f32(inputs["w2"])                  # [C, CC]
    b2 = f32(inputs["b2"])                  # [C]
    w3 = f32(inputs["w3"])                  # [C, C]
    b3 = f32(inputs["b3"])                  # [C]

    wf = w3 @ w2                            # [C, CC] folded weight
    bf = w3 @ b2 + b3                       # [C]     folded bias

    # WfT blocks: pack[p, (j*CI+oi)*P + m] = Wf[oi*P+m, KJ*p+j]
    wft = wf.T.reshape(P, KJ, CI, P).transpose(0, 1, 2, 3)  # [p, j, oi, m]
    pack = np.zeros((P, PACK_COLS), dtype=BF)
    pack[:, : KJ * CI * P] = wft.reshape(P, KJ * CI * P).astype(BF)
    pack[:, OFF_BIAS : OFF_BIAS + CI] = bf.reshape(CI, P).T.astype(BF)

    xb = x.reshape(B, CI, P, S).astype(BF)  # channel = ci*128 + p

    in_maps = []
    for b in range(N_CORES):
        m = {f"x{ci}": xb[b, ci] for ci in range(CI)}
        pkb = pack.copy()
        pkb[:, OFF_CTX : OFF_CTX + KJ] = context[b].reshape(P, KJ).astype(BF)
        m["pack"] = pkb
        in_maps.append(m)
    return in_maps


def run(inputs: dict, trace: bool = False, tmpdir: str | None = None, **build_kw):
    """Build+run on 8 cores; returns (full_output, BassKernelResults)."""
    nc = build_nc(**build_kw)
    in_maps = _prep_in_maps(inputs)
    res = run_bass_kernel_spmd(
        nc, in_maps, list(range(N_CORES)), trace=trace, tmpdir=tmpdir
    )
    out = np.stack(
        [
            np.concatenate(
                [res.results[b][f"out{ci}"] for ci in range(CI)], axis=0
            ).astype(np.float32)
            for b in range(N_CORES)
        ],
        axis=0,
    ).reshape(B, C, H, W)
    return out, res


def kernel(**inputs: np.ndarray) -> np.ndarray:
    out, _ = run(inputs, trace=False)
    return out
